# revision 3
# baseline (speedup 1.0000x reference)
"""Causal multi-head attention on 8 TRN2 NeuronCores.

Reference (per batch b):
    q,k,v = x @ W^T  (W: [d_out, d_in]), split into H=16 heads of dk=64
    attn  = softmax(causal(q k^T / sqrt(dk))) v
    y     = concat_heads(attn) @ W_o^T

Sharding (8 cores): core c -> batch b = c//4, head group g = c%4 (4 heads,
256 channels).  w_q/w_k/w_v column-sharded by head, w_o row-sharded - each
core computes a partial y[b] over its 256 channels; the host sums the 4
partials per batch (the unshard step).  y partials are written bf16 (the
host accumulates in f32; adds ~0.2% to a ~0.56% rel-err budget, gate 2e-2).

Per-core engine load model (bf16, warm 2.4 GHz): PE streams ~270k
column-cycles (~112us: qk-proj 65.5k, v-proj 32.8k, scores 69.6k, PV 69.6k,
y-proj 32.8k), ACT exp ~81us, DVE ~70us.  PE is the bottleneck, so the
kernel is organized to keep its queue dense:

  - Emission interleaves at pair granularity: between each scores-pair and
    its PV (which must wait ~2.3us for ScalarE exp), a "filler" PE group is
    emitted - a projection group for the NEXT s-chunk or an output-proj
    piece for the PREVIOUS chunk.  This hides exp latency and keeps HAM at
    2.4 GHz across chunk boundaries.
  - Head: x chunk 0 is DMAed in 4 pieces on 2 queues, weights are split so
    the et=0 halves of w_k/w_q land first, and the first projection groups
    are emitted in dependency order (k-et0, q-et0, v, with k/q-et1 as
    fillers inside attention chunk 0).
  - Tail: the last chunk's softmax-normalization chain (copy denominator,
    fast reciprocal, gpsimd partition broadcast, multiply) would leave the
    PE idle >3.4us and re-throttle HAM, so dummy f32 matmuls keyed on the
    broadcast output keep it warm; the final y projection then runs at full
    clock with copies alternating DVE/ACT and its DMAs spread over 4 queues.

Layout choices (unchanged from the 160us baseline):
  - All matmul inputs bf16 (host-cast), f32 PSUM accumulation.
  - x staged transposed (x^T: [d, s]) so q^T/k^T ([e_local, s]) come
    straight out of the PE and serve as lhsT/rhs of the scores matmul.
  - Scores computed transposed: S^T[kpos, q], two heads packed in the
    128-row PE array via row tiling, one 2-bank PSUM pair per kpos-tile, so
    a single ScalarE ACTIVATE does exp for both heads.
  - Softmax without max-subtraction (scores are O(10), exp safe in f32).
  - Causality at tile granularity: fully-masked kpos-tiles skipped,
    diagonal tiles shift/shrink to the valid q-range, residual mask is one
    128-col triangle (f >= p) multiply per diagonal tile.
  - P @ V via V_aug = [V | 1]: row 64 of the accumulated [65, q] output is
    the softmax denominator.
  - attn^T = out[0:64] * (1/denominator) via DVE fast reciprocal + gpsimd
    partition broadcast (custom ops need base-partition-0 SBUF inputs).
"""

from functools import partial

import numpy as np
import ml_dtypes

B = 2
S = 2048
D = 1024
H = 16
DK = 64
NCORES = 8
EL = 256  # local channels per core (4 heads)
QW = 512  # q-chunk width (free dim of scores matmuls)
NJ = S // QW  # 4 q-chunks

_CACHE = {}


def _build():
    import concourse.bass as bass
    import concourse.mybir as mybir
    import concourse.tile as tile
    from concourse import bacc

    f32 = mybir.dt.float32
    bf16 = mybir.dt.bfloat16
    ts = bass.ts
    Exp = mybir.ActivationFunctionType.Exp

    nc = bacc.Bacc("TRN2", num_devices=NCORES)
    xT_d = nc.dram_tensor("xT", [D, S], bf16, kind="ExternalInput")
    wqT_d = nc.dram_tensor("wqT", [D, EL], bf16, kind="ExternalInput")
    wkT_d = nc.dram_tensor("wkT", [D, EL], bf16, kind="ExternalInput")
    wvT_d = nc.dram_tensor("wvT", [D, EL], bf16, kind="ExternalInput")
    woT_d = nc.dram_tensor("woT", [EL, D], bf16, kind="ExternalInput")
    y_d = nc.dram_tensor("y", [S, D], bf16, kind="ExternalOutput")

    DT = D // 128  # 8 d-tiles
    ST = S // 128  # 16 s-tiles

    with tile.TileContext(nc) as tc:
        with (
            tc.tile_pool(name="big", bufs=1) as big,
            tc.tile_pool(name="work", bufs=3) as work,
            tc.tile_pool(name="psum", bufs=1, space="PSUM") as psum,
        ):
            xT = big.tile([128, DT, S], bf16)  # x^T (d, s)
            wqT = big.tile([128, DT, EL], bf16)
            wkT = big.tile([128, DT, EL], bf16)
            wvT = big.tile([128, DT, EL], bf16)
            woT = big.tile([128, EL // 128, D], bf16)
            qT = big.tile([128, 2, S], bf16)  # (e_local, s)
            kT = big.tile([128, 2, S], bf16)
            vA = big.tile([128, ST, 4, DK + 1], bf16)  # (s%128, s//128, h, dv+1)
            aT = big.tile([128, 2, S], bf16)  # attn^T (d_local, s)
            masks = big.tile([128, 1, 128], bf16)  # triangle: f >= p
            warmf = big.tile([64, 128], f32)  # lhsT for HAM-keepalive matmuls

            # ---- input DMAs.  x chunk 0 first, 4 pieces on the scalar and
            # gpsimd queues (transfers run in parallel once the DMA engines
            # come up ~8us in); weights on the sync queue with the et=0
            # halves of w_k/w_q leading so the first projection groups are
            # gated only by x chunk 0. ----
            xT_r = xT_d.ap().rearrange("(ko p) s -> p ko s", p=128)
            nc.scalar.dma_start(xT[:, 0:2, ts(0, QW)], xT_r[:, 0:2, ts(0, QW)])
            nc.gpsimd.dma_start(xT[:, 2:4, ts(0, QW)], xT_r[:, 2:4, ts(0, QW)])
            nc.scalar.dma_start(xT[:, 4:6, ts(0, QW)], xT_r[:, 4:6, ts(0, QW)])
            nc.gpsimd.dma_start(xT[:, 6:8, ts(0, QW)], xT_r[:, 6:8, ts(0, QW)])
            wkT_r = wkT_d.ap().rearrange("(ko p) e -> p ko e", p=128)
            wqT_r = wqT_d.ap().rearrange("(ko p) e -> p ko e", p=128)
            nc.sync.dma_start(wkT[:, :, 0:128], wkT_r[:, :, 0:128])
            nc.sync.dma_start(wqT[:, :, 0:128], wqT_r[:, :, 0:128])
            nc.sync.dma_start(
                wvT[:], wvT_d.ap().rearrange("(ko p) e -> p ko e", p=128)
            )
            nc.sync.dma_start(wkT[:, :, 128:EL], wkT_r[:, :, 128:EL])
            nc.sync.dma_start(wqT[:, :, 128:EL], wqT_r[:, :, 128:EL])
            for sh in range(1, NJ):
                nc.sync.dma_start(xT[:, :, ts(sh, QW)], xT_r[:, :, ts(sh, QW)])
            nc.sync.dma_start(
                woT[:], woT_d.ap().rearrange("(ko p) e -> p ko e", p=128)
            )

            # ---- constants (after the DMA issues so those queues start the
            # x loads first): triangle mask + V_aug ones + warm lhsT ----
            # mask[p, f] = 1.0 iff f >= p
            nc.gpsimd.memset(masks[:], 1.0)
            nc.gpsimd.affine_select(
                out=masks[:, 0, :],
                in_=masks[:, 0, :],
                compare_op=mybir.AluOpType.is_ge,
                fill=0.0,
                base=0,
                pattern=[[1, 128]],
                channel_multiplier=-1,
            )
            nc.gpsimd.memset(vA[:, :, :, DK], 1.0)
            nc.vector.memset(warmf[:], 0.0)

            # ---- PE warmup: dummy matmuls on zeroed scratch during the DMA
            # wait so HAM ramps toward 2.4 GHz before real matmuls arrive.
            warm = work.tile([128, QW], bf16, tag="warm", bufs=1)
            nc.vector.memset(warm[:], 0.0)
            for g in range(10):
                wp = psum.tile([128, QW], f32, tag="sc", bufs=2)
                nc.tensor.matmul(wp[:], warm[:, 0:128], warm[:])

            def qk_group(sj, which, et):
                w_sb, outT = (wkT, kT) if which == 0 else (wqT, qT)
                ps = psum.tile([128, QW], f32, tag="mm", bufs=2)
                for kd in range(DT):
                    nc.tensor.matmul(
                        ps[:],
                        w_sb[:, kd, ts(et, 128)],
                        xT[:, kd, ts(sj, QW)],
                        start=(kd == 0),
                        stop=(kd == DT - 1),
                    )
                nc.vector.tensor_copy(outT[:, et, ts(sj, QW)], ps[:])

            def v_group(st):
                ps = psum.tile([128, EL], f32, tag="mm", bufs=2)
                for kd in range(DT):
                    nc.tensor.matmul(
                        ps[:],
                        xT[:, kd, ts(st, 128)],
                        wvT[:, kd, :],
                        start=(kd == 0),
                        stop=(kd == DT - 1),
                    )
                nc.vector.tensor_copy(
                    vA[:, st, :, 0:DK],
                    ps[:].rearrange("p (h e) -> p h e", h=4),
                )

            y_r = y_d.ap().rearrange("(so p) e -> p so e", p=128)
            dmaq = [nc.sync, nc.scalar, nc.gpsimd, nc.sync]

            def sd_group(jd, st, eo, alt=0):
                # one (s-tile, 512-col) piece of y = attn^T.T @ w_o^T
                ps = psum.tile([128, QW], f32, tag="mm", bufs=2)
                for kd in range(EL // 128):
                    nc.tensor.matmul(
                        ps[:],
                        aT[:, kd, ts(st, 128)],
                        woT[:, kd, ts(eo, QW)],
                        start=(kd == 0),
                        stop=(kd == EL // 128 - 1),
                    )
                yt = work.tile([128, QW], bf16, tag="yout", bufs=4)
                # tail pieces alternate DVE/ACT (both idle there) and spread
                # their DMAs over 4 queues; mid-kernel pieces stay on
                # DVE+sync to keep the exp stream clean
                if alt % 2 == 1:
                    nc.scalar.copy(yt[:], ps[:])
                else:
                    nc.vector.tensor_copy(yt[:], ps[:])
                q = dmaq[alt % 4] if jd == NJ - 1 else nc.sync
                q.dma_start(y_r[:, st, ts(eo, QW)], yt[:])

            fillers = []

            def run_filler():
                if fillers:
                    fillers.pop(0)()

            def attn_chunk(j):
                last = j == NJ - 1
                ilast = 4 * j + 3
                for hp in range(2):
                    oa0 = psum.tile([128, QW], f32, tag="oa", bufs=2)
                    oa1 = psum.tile([128, QW], f32, tag="oa", bufs=2)

                    def geom(i):
                        # diagonal tiles only need q >= kpos: shift the
                        # q-range by 128r and shrink; the remaining mask is
                        # always the 128-col triangle f>=p.
                        if i >= 4 * j:
                            off = 128 * (i - 4 * j)
                            return off, QW - off
                        return 0, QW

                    for ii in range(0, ilast + 1, 2):
                        pair = (ii, ii + 1)
                        scs = {}
                        for i in pair:
                            off, NW = geom(i)
                            sc = psum.tile([128, 2 * QW], f32, tag="sc", bufs=2)
                            nc.tensor.matmul(
                                sc[:, 0:NW],
                                kT[0:64, hp, ts(i, 128)],
                                qT[0:64, hp, bass.ds(j * QW + off, NW)],
                                tile_position=(0, 0),
                            )
                            nc.tensor.matmul(
                                sc[:, QW : QW + NW],
                                kT[64:128, hp, ts(i, 128)],
                                qT[64:128, hp, bass.ds(j * QW + off, NW)],
                                tile_position=(64, 0),
                            )
                            scs[i] = sc
                        # filler PE group lands between the scores pair and
                        # its PV, hiding the ScalarE exp latency
                        run_filler()
                        eos = {}
                        for i in pair:
                            off, NW = geom(i)
                            e01 = work.tile(
                                [128, 2 * QW], bf16, tag="exps", bufs=8
                            )
                            sc_v = scs[i][:].rearrange("p (h q) -> p h q", h=2)
                            e_v = e01[:].rearrange("p (h q) -> p h q", h=2)
                            nc.scalar.activation(
                                e_v[:, :, 0:NW], sc_v[:, :, 0:NW], Exp, scale=0.125
                            )
                            if i >= 4 * j:  # diagonal: mask first 128 cols
                                nc.vector.tensor_mul(
                                    e_v[:, :, 0:128],
                                    e_v[:, :, 0:128],
                                    masks[:, 0:1, :].to_broadcast((128, 2, 128)),
                                )
                            eos[i] = e01
                        for i in pair:
                            off, NW = geom(i)
                            nc.tensor.matmul(
                                oa0[0 : DK + 1, off : off + NW],
                                vA[:, i, 2 * hp, :],
                                eos[i][:, 0:NW],
                                start=(i == 0),
                                stop=(i == ilast),
                            )
                            nc.tensor.matmul(
                                oa1[0 : DK + 1, off : off + NW],
                                vA[:, i, 2 * hp + 1, :],
                                eos[i][:, QW : QW + NW],
                                start=(i == 0),
                                stop=(i == ilast),
                            )
                    # softmax normalization; on the very last chunk, dummy
                    # f32 matmuls keyed on the broadcasts keep HAM warm
                    # through the otherwise PE-idle reciprocal chain
                    bcs = {}
                    for hh, oa in ((0, oa0), (1, oa1)):
                        dn = work.tile([1, QW], f32, tag="dn", bufs=3)
                        nc.vector.tensor_copy(dn[:], oa[DK : DK + 1, :])
                        rc = work.tile([1, QW], f32, tag="rc", bufs=3)
                        nc.vector.reciprocal_approx_fast(out=rc[:], in_=dn[:])
                        bc = work.tile([64, QW], f32, tag="bc", bufs=3)
                        nc.gpsimd.partition_broadcast(bc[:], rc[:])
                        bcs[hh] = bc
                        if last and hp == 1:
                            wp = psum.tile([128, QW], f32, tag="sc", bufs=2)
                            nc.tensor.matmul(wp[:], warmf[:], bc[:])
                    for hh, oa in ((0, oa0), (1, oa1)):
                        h = 2 * hp + hh
                        nc.vector.tensor_mul(
                            aT[(h % 2) * 64 : (h % 2) * 64 + 64, h // 2, ts(j, QW)],
                            oa[0:DK, :],
                            bcs[hh][:],
                        )

            # ---- the pipeline ----
            # chunk 0 projections in dependency order; k/q et=1 become the
            # first fillers (attention chunk 0 hp=0 only needs et=0)
            qk_group(0, 0, 0)
            qk_group(0, 1, 0)
            for st in range(4):
                v_group(st)
            fillers.append(partial(qk_group, 0, 0, 1))
            fillers.append(partial(qk_group, 0, 1, 1))

            for sj in range(NJ):
                # queue filler jobs consumed inside attn_chunk(sj): the
                # next chunk's projections (k/q et0 first - attn(sj+1) hp0
                # needs them) interleaved with the previous chunk's output
                # projection pieces
                jobs = []
                if sj + 1 < NJ:
                    jobs += [
                        partial(qk_group, sj + 1, 0, 0),
                        partial(qk_group, sj + 1, 1, 0),
                        partial(v_group, 4 * (sj + 1) + 0),
                        partial(v_group, 4 * (sj + 1) + 1),
                        partial(qk_group, sj + 1, 0, 1),
                        partial(qk_group, sj + 1, 1, 1),
                        partial(v_group, 4 * (sj + 1) + 2),
                        partial(v_group, 4 * (sj + 1) + 3),
                    ]
                if sj >= 1:
                    sd = [
                        partial(sd_group, sj - 1, 4 * (sj - 1) + st, eo)
                        for st in range(4)
                        for eo in range(2)
                    ]
                    # interleave: proj jobs lead (longest dependency chain)
                    mixed = []
                    for a, b in zip(jobs, sd):
                        mixed += [a, b]
                    mixed += jobs[len(sd) :] + sd[len(jobs) :]
                    jobs = mixed
                fillers.extend(jobs)
                attn_chunk(sj)
                while fillers:
                    run_filler()
            for idx, (st, eo) in enumerate(
                [(st, eo) for st in range(4 * (NJ - 1), 4 * NJ) for eo in range(2)]
            ):
                sd_group(NJ - 1, st, eo, alt=idx)

    nc.compile()
    return nc


def _get_nc():
    if "nc" not in _CACHE:
        _CACHE["nc"] = _build()
    return _CACHE["nc"]


def kernel(x, w_q, w_k, w_v, w_o, _trace=False, _trace_cores=None):
    from concourse.bass_utils import run_bass_kernel_spmd

    nc = _get_nc()
    bf = ml_dtypes.bfloat16
    in_maps = []
    for c in range(NCORES):
        b = c // 4
        g = c % 4
        ch = slice(g * EL, (g + 1) * EL)
        in_maps.append(
            {
                "xT": np.ascontiguousarray(x[b].T).astype(bf),
                "wqT": np.ascontiguousarray(w_q[ch, :].T).astype(bf),
                "wkT": np.ascontiguousarray(w_k[ch, :].T).astype(bf),
                "wvT": np.ascontiguousarray(w_v[ch, :].T).astype(bf),
                "woT": np.ascontiguousarray(w_o[:, ch].T).astype(bf),
            }
        )
    res = run_bass_kernel_spmd(
        nc,
        in_maps,
        core_ids=list(range(NCORES)),
        trace=_trace,
        trace_cores=_trace_cores,
    )
    _CACHE["last_results"] = res
    y = np.zeros((B, S, D), np.float32)
    for c in range(NCORES):
        y[c // 4] += res.results[c]["y"].astype(np.float32)
    return y
